# revision 11
# baseline (speedup 1.0000x reference)
"""Bag-of-words histogram kernel for Trainium2 (Bass/Tile), 8-core data-parallel.

Problem: docs [256, 2048] int32 token ids in [0, 32000) ->
         hist [256, 32000] fp32, hist[b, v] = count(docs[b, :] == v) / 2048.

Algorithm (per core, 32 rows):
  Factor each token t = 256*hi + lo (hi < 125, lo < 256). Then
    hist[b, hi, lo] = sum_s onehot_hi[s, hi] * onehot_lo[s, lo]
  i.e. a [128-token] x [128 hi] x [256 lo] outer-product matmul accumulated
  over 16 K-tiles per row on the PE.

  One-hot builds (measured-on-HW design):
  - hi: ONE tensor_tensor is_equal per row builds all 16 k-tiles in a
    k-interleaved [128, 128 hi, 16 k] tile (inner stride-1 on both operands
    keeps the DVE 2x mode; ~1.3us/row vs ~3us for 16 tensor_scalars). The
    matmul reads lhsT through a stride-16 AP, which the PE weight-load path
    tolerates (+34ns/matmul measured).
  - lo: per-k-tile tensor_scalar is_equal (4x DVE mode; the moving rhs must
    stay contiguous - strided rhs is 4x slower on the PE). A tunable subset
    of k-tiles is built on the scalar engine (Square+Relu pair) to offload
    the DVE bottleneck.
  - The exact 2^-11 output scaling rides on the lo one-hot values (op1 mult
    / Relu bias), so PSUM holds the final scaled histogram and is DMA'd
    straight to HBM with no ACT copyback.

Sharding: batch axis split 8 ways (32 rows per core), no communication.
"""

import sys

import numpy as np

for _p in ("/opt/trn_rl_repo",):
    if _p not in sys.path:
        sys.path.append(_p)

BATCH = 256
SEQ = 2048
VOCAB = 32000
N_CORES = 8
ROWS = BATCH // N_CORES  # 32 rows per core
P = 128
KT = SEQ // P            # 16 K-tiles per row
NLO = 256                # low-digit bins (t & 255)
NHI = 128                # high-digit compare width (t >> 8 < 125)
SCALE = 2.0 ** -11       # 1/SEQ, exact in bf16

# k-tiles whose lo one-hot is built off the DVE. ACT pair ~800ns, gpsimd
# ~2.3us (but otherwise idle); both are emitted one row ahead and consumed
# at the END of the row's PSUM chain so their latency hides behind the DVE
# k-tiles. Tuned from HW runs: DVE lo-build ~218ns.
ACT_K = (3, 7, 11, 15)
GPS_K = (0,)
DVE_K = tuple(k for k in range(KT) if k not in ACT_K and k not in GPS_K)
CHAIN = DVE_K + GPS_K + ACT_K  # PSUM accumulation order


def _build_nc():
    from contextlib import ExitStack

    from concourse import bacc, bass, mybir
    from concourse.tile import TileContext

    nc = bacc.Bacc()
    docs = nc.dram_tensor("docs", [ROWS, SEQ], mybir.dt.int32, kind="ExternalInput")
    hist = nc.dram_tensor("hist", [ROWS, VOCAB], mybir.dt.float32, kind="ExternalOutput")

    f32 = mybir.dt.float32
    bf16 = mybir.dt.bfloat16
    Alu = mybir.AluOpType
    Act = mybir.ActivationFunctionType

    with TileContext(nc) as tc, ExitStack() as ctx:
        const_tp = ctx.enter_context(tc.tile_pool(name="const", bufs=1))
        ohhi_tp = ctx.enter_context(tc.tile_pool(name="ohhi", bufs=4))
        ohlo_tp = ctx.enter_context(tc.tile_pool(name="ohlo", bufs=32))
        sq_tp = ctx.enter_context(tc.tile_pool(name="sq", bufs=8))
        res_tp = ctx.enter_context(tc.tile_pool(name="res", bufs=4))
        psum_tp = ctx.enter_context(tc.tile_pool(name="psum", bufs=8, space="PSUM"))

        # SCALE as a per-partition const AP (f32 immediates for scalar2/bias
        # are not supported by the const-AP lowering)
        scale_c = const_tp.tile([P, 1], f32)
        nc.vector.memset(scale_c[:], SCALE)

        # iota constants
        iota_lo = const_tp.tile([P, NLO], bf16)
        nc.gpsimd.iota(iota_lo[:], [[1, NLO]], channel_multiplier=0,
                       allow_small_or_imprecise_dtypes=True)
        # k-interleaved hi iota: value = h at [p, h, k] for all k
        iota_hi16 = const_tp.tile([P, NHI, KT], bf16)
        nc.gpsimd.iota(iota_hi16[:], [[1, NHI], [0, KT]], channel_multiplier=0,
                       allow_small_or_imprecise_dtypes=True)

        # Load all 32 rows in one DMA; partition p holds tokens [16p, 16p+16)
        # of each row (within-row permutation is histogram-invariant).
        tok = const_tp.tile([P, ROWS, KT], mybir.dt.int32)
        nc.sync.dma_start(out=tok[:],
                          in_=bass.AP(docs, 0, [[16, P], [SEQ, ROWS], [1, KT]]))

        #

        # hi = t >> 8 (bf16 for the 2x tensor_tensor), lo = t & 255
        # (f32 scalar + negated f32 for the ACT bias path).
        hi_i = const_tp.tile([P, ROWS, KT], mybir.dt.int32)
        lo_i = const_tp.tile([P, ROWS, KT], mybir.dt.int32)
        nc.vector.tensor_scalar(out=hi_i[:], in0=tok[:], scalar1=8,
                                scalar2=None, op0=Alu.logical_shift_right)
        nc.vector.tensor_scalar(out=lo_i[:], in0=tok[:], scalar1=255,
                                scalar2=None, op0=Alu.bitwise_and)
        hi_bf = const_tp.tile([P, ROWS, KT], bf16)
        lo_f = const_tp.tile([P, ROWS, KT], f32)
        nlo_f = const_tp.tile([P, ROWS, KT], f32)
        nc.vector.tensor_scalar(out=hi_bf[:], in0=hi_i[:], scalar1=1.0,
                                scalar2=None, op0=Alu.mult)
        nc.vector.tensor_scalar(out=lo_f[:], in0=lo_i[:], scalar1=1.0,
                                scalar2=None, op0=Alu.mult)
        nc.vector.tensor_scalar(out=nlo_f[:], in0=lo_i[:], scalar1=-1.0,
                                scalar2=None, op0=Alu.mult)

        def build_act(r):
            # (iota - lo)^2 then 2^-11 * relu(1 - d2): exact scaled one-hot
            # on the scalar engine
            tiles = {}
            for k in ACT_K:
                sq = sq_tp.tile([P, NLO], bf16, tag="sq")
                ohlo = ohlo_tp.tile([P, NLO], bf16, tag="ohlo_a")
                nc.scalar.activation(
                    out=sq[:], in_=iota_lo[:], func=Act.Square,
                    bias=nlo_f[:, r, k:k + 1], scale=1.0)
                nc.scalar.activation(
                    out=ohlo[:], in_=sq[:], func=Act.Relu,
                    bias=scale_c[:, 0:1], scale=-SCALE)
                tiles[k] = ohlo
            return tiles

        def build_gps(r):
            tiles = {}
            for k in GPS_K:
                ohlo = ohlo_tp.tile([P, NLO], bf16, tag="ohlo_g")
                nc.gpsimd.tensor_scalar(
                    out=ohlo[:], in0=iota_lo[:],
                    scalar1=lo_f[:, r, k:k + 1], scalar2=scale_c[:, 0:1],
                    op0=Alu.is_equal, op1=Alu.mult)
                tiles[k] = ohlo
            return tiles

        def build_hi(r):
            # all 16 k-tiles' hi one-hots in one DVE op
            ohhi16 = ohhi_tp.tile([P, NHI, KT], bf16)
            nc.vector.tensor_tensor(
                out=ohhi16[:], in0=iota_hi16[:],
                in1=hi_bf[:, r, :].unsqueeze(1).broadcast_to([P, NHI, KT]),
                op=Alu.is_equal)
            return ohhi16

        # software pipeline: slow engines build row r+1 while the PE chews
        # row r; their tiles are consumed at the end of row r+1's chain
        hi_t = build_hi(0)
        act_t = build_act(0)
        gps_t = build_gps(0)
        for r in range(ROWS):
            ps = psum_tp.tile([P, NLO], f32)
            off_t = {**act_t, **gps_t}
            hi_cur = hi_t
            for i, k in enumerate(CHAIN):
                if k in off_t:
                    ohlo = off_t[k]
                else:
                    ohlo = ohlo_tp.tile([P, NLO], bf16, tag="ohlo")
                    nc.vector.tensor_scalar(
                        out=ohlo[:], in0=iota_lo[:],
                        scalar1=lo_f[:, r, k:k + 1], scalar2=scale_c[:, 0:1],
                        op0=Alu.is_equal, op1=Alu.mult)
                if i == len(DVE_K) - 1 and r + 1 < ROWS:
                    # DVE work for row r done; queue next row's builds
                    hi_t = build_hi(r + 1)
                    act_t = build_act(r + 1)
                    gps_t = build_gps(r + 1)
                nc.tensor.matmul(out=ps[:], lhsT=hi_cur[:, :, k], rhs=ohlo[:],
                                 start=(i == 0), stop=(i == KT - 1))

            # PSUM holds hist/2048 (gpsimd cannot read PSUM)
            res = res_tp.tile([P, NLO], f32)
            nc.scalar.mul(out=res[:], in_=ps[:], mul=1.0)
            nc.sync.dma_start(
                out=hist[r].rearrange("(h l) -> h l", l=NLO),
                in_=res[:VOCAB // NLO, :])
    nc.compile()
    return nc


_NC_CACHE = None


def _get_nc():
    global _NC_CACHE
    if _NC_CACHE is None:
        _NC_CACHE = _build_nc()
    return _NC_CACHE


def run_sharded(docs: np.ndarray, trace: bool = False):
    """Run the 8-core SPMD kernel. Returns (full_output, BassKernelResults)."""
    from concourse.bass_utils import run_bass_kernel_spmd

    docs = np.ascontiguousarray(np.asarray(docs, dtype=np.int32))
    assert docs.shape == (BATCH, SEQ), docs.shape
    shards = docs.reshape(N_CORES, ROWS, SEQ)
    in_maps = [{"docs": shards[i]} for i in range(N_CORES)]
    res = run_bass_kernel_spmd(_get_nc(), in_maps, core_ids=list(range(N_CORES)),
                               trace=trace)
    out = np.concatenate([res.results[i]["hist"] for i in range(N_CORES)], axis=0)
    return out, res


def kernel(docs: np.ndarray) -> np.ndarray:
    out, _ = run_sharded(docs, trace=False)
    return out


# revision 12
# speedup vs baseline: 1.9310x; 1.9310x over previous
"""Bag-of-words histogram kernel for Trainium2 (Bass/Tile), 8-core data-parallel.

Problem: docs [256, 2048] int32 token ids in [0, 32000) ->
         hist [256, 32000] fp32, hist[b, v] = count(docs[b, :] == v) / 2048.

Algorithm (per core, 32 rows):
  Factor each token t = 256*hi + lo (hi < 125, lo < 256). Then
    hist[b, hi, lo] = sum_s onehot_hi[s, hi] * onehot_lo[s, lo]
  i.e. a [128-token] x [128 hi] x [256 lo] outer-product matmul accumulated
  over 16 K-tiles per row on the PE.

  One-hot builds (measured-on-HW design):
  - hi: ONE tensor_tensor is_equal per row builds all 16 k-tiles in a
    k-interleaved [128, 128 hi, 16 k] tile (inner stride-1 on both operands
    keeps the DVE 2x mode; ~1.3us/row vs ~3us for 16 tensor_scalars). The
    matmul reads lhsT through a stride-16 AP, which the PE weight-load path
    tolerates (+34ns/matmul measured).
  - lo: per-k-tile tensor_scalar is_equal (4x DVE mode; the moving rhs must
    stay contiguous - strided rhs is 4x slower on the PE). A tunable subset
    of k-tiles is built on the scalar engine (Square+Relu pair) to offload
    the DVE bottleneck.
  - The exact 2^-11 output scaling rides on the lo one-hot values (op1 mult
    / Relu bias), so PSUM holds the final scaled histogram and is DMA'd
    straight to HBM with no ACT copyback.

Sharding: batch axis split 8 ways (32 rows per core), no communication.
"""

import sys

import numpy as np

for _p in ("/opt/trn_rl_repo",):
    if _p not in sys.path:
        sys.path.append(_p)

BATCH = 256
SEQ = 2048
VOCAB = 32000
N_CORES = 8
ROWS = BATCH // N_CORES  # 32 rows per core
P = 128
KT = SEQ // P            # 16 K-tiles per row
NLO = 256                # low-digit bins (t & 255)
NHI = 128                # high-digit compare width (t >> 8 < 125)
SCALE = 2.0 ** -11       # 1/SEQ, exact in bf16

# k-tiles whose lo one-hot is built off the DVE. ACT pair ~800ns, gpsimd
# ~2.3us (but otherwise idle); both are emitted one row ahead and consumed
# at the END of the row's PSUM chain so their latency hides behind the DVE
# k-tiles. Tuned from HW runs: DVE lo-build ~218ns.
ACT_K = (3, 7, 11, 15)
GPS_K = ()
DVE_K = tuple(k for k in range(KT) if k not in ACT_K and k not in GPS_K)
CHAIN = DVE_K + GPS_K + ACT_K  # PSUM accumulation order


def _build_nc():
    from contextlib import ExitStack

    from concourse import bacc, bass, mybir
    from concourse.tile import TileContext

    nc = bacc.Bacc()
    docs = nc.dram_tensor("docs", [ROWS, SEQ], mybir.dt.int32, kind="ExternalInput")
    hist = nc.dram_tensor("hist", [ROWS, VOCAB], mybir.dt.float32, kind="ExternalOutput")

    f32 = mybir.dt.float32
    bf16 = mybir.dt.bfloat16
    Alu = mybir.AluOpType
    Act = mybir.ActivationFunctionType

    with TileContext(nc) as tc, ExitStack() as ctx:
        const_tp = ctx.enter_context(tc.tile_pool(name="const", bufs=1))
        ohhi_tp = ctx.enter_context(tc.tile_pool(name="ohhi", bufs=4))
        ohlo_tp = ctx.enter_context(tc.tile_pool(name="ohlo", bufs=32))
        sq_tp = ctx.enter_context(tc.tile_pool(name="sq", bufs=8))
        res_tp = ctx.enter_context(tc.tile_pool(name="res", bufs=4))
        psum_tp = ctx.enter_context(tc.tile_pool(name="psum", bufs=8, space="PSUM"))

        # SCALE as a per-partition const AP (f32 immediates for scalar2/bias
        # are not supported by the const-AP lowering)
        scale_c = const_tp.tile([P, 1], f32)
        nc.vector.memset(scale_c[:], SCALE)

        # iota constants
        iota_lo = const_tp.tile([P, NLO], bf16)
        nc.gpsimd.iota(iota_lo[:], [[1, NLO]], channel_multiplier=0,
                       allow_small_or_imprecise_dtypes=True)
        # k-interleaved hi iota: value = h at [p, h, k] for all k
        iota_hi16 = const_tp.tile([P, NHI, KT], bf16)
        nc.gpsimd.iota(iota_hi16[:], [[1, NHI], [0, KT]], channel_multiplier=0,
                       allow_small_or_imprecise_dtypes=True)

        # Load all 32 rows in one DMA; partition p holds tokens [16p, 16p+16)
        # of each row (within-row permutation is histogram-invariant).
        tok = const_tp.tile([P, ROWS, KT], mybir.dt.int32)
        nc.sync.dma_start(out=tok[:],
                          in_=bass.AP(docs, 0, [[16, P], [SEQ, ROWS], [1, KT]]))

        #

        # hi = t >> 8 (bf16 for the 2x tensor_tensor), lo = t & 255
        # (f32 scalar + negated f32 for the ACT bias path).
        hi_i = const_tp.tile([P, ROWS, KT], mybir.dt.int32)
        lo_i = const_tp.tile([P, ROWS, KT], mybir.dt.int32)
        nc.vector.tensor_scalar(out=hi_i[:], in0=tok[:], scalar1=8,
                                scalar2=None, op0=Alu.logical_shift_right)
        nc.vector.tensor_scalar(out=lo_i[:], in0=tok[:], scalar1=255,
                                scalar2=None, op0=Alu.bitwise_and)
        hi_bf = const_tp.tile([P, ROWS, KT], bf16)
        lo_f = const_tp.tile([P, ROWS, KT], f32)
        nlo_f = const_tp.tile([P, ROWS, KT], f32)
        nc.vector.tensor_scalar(out=hi_bf[:], in0=hi_i[:], scalar1=1.0,
                                scalar2=None, op0=Alu.mult)
        nc.vector.tensor_scalar(out=lo_f[:], in0=lo_i[:], scalar1=1.0,
                                scalar2=None, op0=Alu.mult)
        nc.vector.tensor_scalar(out=nlo_f[:], in0=lo_i[:], scalar1=-1.0,
                                scalar2=None, op0=Alu.mult)

        def build_act(r):
            # (iota - lo)^2 then 2^-11 * relu(1 - d2): exact scaled one-hot
            # on the scalar engine
            tiles = {}
            for k in ACT_K:
                sq = sq_tp.tile([P, NLO], bf16, tag="sq")
                ohlo = ohlo_tp.tile([P, NLO], bf16, tag="ohlo_a")
                nc.scalar.activation(
                    out=sq[:], in_=iota_lo[:], func=Act.Square,
                    bias=nlo_f[:, r, k:k + 1], scale=1.0)
                nc.scalar.activation(
                    out=ohlo[:], in_=sq[:], func=Act.Relu,
                    bias=scale_c[:, 0:1], scale=-SCALE)
                tiles[k] = ohlo
            return tiles

        def build_gps(r):
            tiles = {}
            for k in GPS_K:
                ohlo = ohlo_tp.tile([P, NLO], bf16, tag="ohlo_g")
                nc.gpsimd.tensor_scalar(
                    out=ohlo[:], in0=iota_lo[:],
                    scalar1=lo_f[:, r, k:k + 1], scalar2=scale_c[:, 0:1],
                    op0=Alu.is_equal, op1=Alu.mult)
                tiles[k] = ohlo
            return tiles

        def build_hi(r):
            # all 16 k-tiles' hi one-hots in one DVE op
            ohhi16 = ohhi_tp.tile([P, NHI, KT], bf16)
            nc.vector.tensor_tensor(
                out=ohhi16[:], in0=iota_hi16[:],
                in1=hi_bf[:, r, :].unsqueeze(1).broadcast_to([P, NHI, KT]),
                op=Alu.is_equal)
            return ohhi16

        # software pipeline: slow engines build row r+1 while the PE chews
        # row r; their tiles are consumed at the end of row r+1's chain
        hi_t = build_hi(0)
        act_t = build_act(0)
        gps_t = build_gps(0)
        for r in range(ROWS):
            ps = psum_tp.tile([P, NLO], f32)
            off_t = {**act_t, **gps_t}
            hi_cur = hi_t
            for i, k in enumerate(CHAIN):
                if k in off_t:
                    ohlo = off_t[k]
                else:
                    ohlo = ohlo_tp.tile([P, NLO], bf16, tag="ohlo")
                    nc.vector.tensor_scalar(
                        out=ohlo[:], in0=iota_lo[:],
                        scalar1=lo_f[:, r, k:k + 1], scalar2=scale_c[:, 0:1],
                        op0=Alu.is_equal, op1=Alu.mult)
                if i == len(DVE_K) - 1 and r + 1 < ROWS:
                    # DVE work for row r done; queue next row's builds
                    hi_t = build_hi(r + 1)
                    act_t = build_act(r + 1)
                    gps_t = build_gps(r + 1)
                nc.tensor.matmul(out=ps[:], lhsT=hi_cur[:, :, k], rhs=ohlo[:],
                                 start=(i == 0), stop=(i == KT - 1))

            # PSUM holds hist/2048 (gpsimd cannot read PSUM)
            res = res_tp.tile([P, NLO], f32)
            nc.scalar.mul(out=res[:], in_=ps[:], mul=1.0)
            nc.sync.dma_start(
                out=hist[r].rearrange("(h l) -> h l", l=NLO),
                in_=res[:VOCAB // NLO, :])
    nc.compile()
    return nc


_NC_CACHE = None


def _get_nc():
    global _NC_CACHE
    if _NC_CACHE is None:
        _NC_CACHE = _build_nc()
    return _NC_CACHE


def run_sharded(docs: np.ndarray, trace: bool = False):
    """Run the 8-core SPMD kernel. Returns (full_output, BassKernelResults)."""
    from concourse.bass_utils import run_bass_kernel_spmd

    docs = np.ascontiguousarray(np.asarray(docs, dtype=np.int32))
    assert docs.shape == (BATCH, SEQ), docs.shape
    shards = docs.reshape(N_CORES, ROWS, SEQ)
    in_maps = [{"docs": shards[i]} for i in range(N_CORES)]
    res = run_bass_kernel_spmd(_get_nc(), in_maps, core_ids=list(range(N_CORES)),
                               trace=trace)
    out = np.concatenate([res.results[i]["hist"] for i in range(N_CORES)], axis=0)
    return out, res


def kernel(docs: np.ndarray) -> np.ndarray:
    out, _ = run_sharded(docs, trace=False)
    return out


# revision 14
# speedup vs baseline: 2.0472x; 1.0602x over previous
"""Bag-of-words histogram kernel for Trainium2 (Bass/Tile), 8-core data-parallel.

Problem: docs [256, 2048] int32 token ids in [0, 32000) ->
         hist [256, 32000] fp32, hist[b, v] = count(docs[b, :] == v) / 2048.

Algorithm (per core, 32 rows):
  Factor each token t = 256*hi + lo (hi < 125, lo < 256). Then
    hist[b, hi, lo] = sum_s onehot_hi[s, hi] * onehot_lo[s, lo]
  i.e. a [128-token] x [128 hi] x [256 lo] outer-product matmul accumulated
  over 16 K-tiles per row on the PE.

  One-hot builds (measured-on-HW design):
  - hi: ONE tensor_tensor is_equal per row builds all 16 k-tiles in a
    k-interleaved [128, 128 hi, 16 k] tile (inner stride-1 on both operands
    keeps the DVE 2x mode; ~1.3us/row vs ~3us for 16 tensor_scalars). The
    matmul reads lhsT through a stride-16 AP, which the PE weight-load path
    tolerates (+34ns/matmul measured).
  - lo: per-k-tile tensor_scalar is_equal (4x DVE mode; the moving rhs must
    stay contiguous - strided rhs is 4x slower on the PE). A tunable subset
    of k-tiles is built on the scalar engine (Square+Relu pair) to offload
    the DVE bottleneck.
  - The exact 2^-11 output scaling rides on the lo one-hot values (op1 mult
    / Relu bias), so PSUM holds the final scaled histogram and is DMA'd
    straight to HBM with no ACT copyback.

Sharding: batch axis split 8 ways (32 rows per core), no communication.
"""

import sys

import numpy as np

for _p in ("/opt/trn_rl_repo",):
    if _p not in sys.path:
        sys.path.append(_p)

BATCH = 256
SEQ = 2048
VOCAB = 32000
N_CORES = 8
ROWS = BATCH // N_CORES  # 32 rows per core
P = 128
KT = SEQ // P            # 16 K-tiles per row
NLO = 256                # low-digit bins (t & 255)
NHI = 128                # high-digit compare width (t >> 8 < 125)
SCALE = 2.0 ** -11       # 1/SEQ, exact in bf16

# k-tiles whose lo one-hot is built off the DVE. ACT pair ~800ns, gpsimd
# ~2.3us (but otherwise idle); both are emitted one row ahead and consumed
# at the END of the row's PSUM chain so their latency hides behind the DVE
# k-tiles. Tuned from HW runs: DVE lo-build ~218ns.
ACT_K = (1, 4, 7, 10, 13)
GPS_K = ()
DVE_K = tuple(k for k in range(KT) if k not in ACT_K and k not in GPS_K)
CHAIN = DVE_K + GPS_K + ACT_K  # PSUM accumulation order


def _build_nc():
    from contextlib import ExitStack

    from concourse import bacc, bass, mybir
    from concourse.tile import TileContext

    nc = bacc.Bacc()
    docs = nc.dram_tensor("docs", [ROWS, SEQ], mybir.dt.int32, kind="ExternalInput")
    hist = nc.dram_tensor("hist", [ROWS, VOCAB], mybir.dt.float32, kind="ExternalOutput")

    f32 = mybir.dt.float32
    bf16 = mybir.dt.bfloat16
    Alu = mybir.AluOpType
    Act = mybir.ActivationFunctionType

    with TileContext(nc) as tc, ExitStack() as ctx:
        const_tp = ctx.enter_context(tc.tile_pool(name="const", bufs=1))
        ohhi_tp = ctx.enter_context(tc.tile_pool(name="ohhi", bufs=4))
        ohlo_tp = ctx.enter_context(tc.tile_pool(name="ohlo", bufs=32))
        sq_tp = ctx.enter_context(tc.tile_pool(name="sq", bufs=8))
        res_tp = ctx.enter_context(tc.tile_pool(name="res", bufs=4))
        psum_tp = ctx.enter_context(tc.tile_pool(name="psum", bufs=8, space="PSUM"))

        # SCALE as a per-partition const AP (f32 immediates for scalar2/bias
        # are not supported by the const-AP lowering)
        scale_c = const_tp.tile([P, 1], f32)
        nc.vector.memset(scale_c[:], SCALE)

        # iota constants
        iota_lo = const_tp.tile([P, NLO], bf16)
        nc.gpsimd.iota(iota_lo[:], [[1, NLO]], channel_multiplier=0,
                       allow_small_or_imprecise_dtypes=True)
        # k-interleaved hi iota: value = h at [p, h, k] for all k
        iota_hi16 = const_tp.tile([P, NHI, KT], bf16)
        nc.gpsimd.iota(iota_hi16[:], [[1, NHI], [0, KT]], channel_multiplier=0,
                       allow_small_or_imprecise_dtypes=True)

        # Load all 32 rows in one DMA; partition p holds tokens [16p, 16p+16)
        # of each row (within-row permutation is histogram-invariant).
        tok = const_tp.tile([P, ROWS, KT], mybir.dt.int32)
        nc.sync.dma_start(out=tok[:],
                          in_=bass.AP(docs, 0, [[16, P], [SEQ, ROWS], [1, KT]]))

        #

        # hi = t >> 8 (bf16 for the 2x tensor_tensor), lo = t & 255
        # (f32 scalar + negated f32 for the ACT bias path).
        hi_i = const_tp.tile([P, ROWS, KT], mybir.dt.int32)
        lo_i = const_tp.tile([P, ROWS, KT], mybir.dt.int32)
        nc.vector.tensor_scalar(out=hi_i[:], in0=tok[:], scalar1=8,
                                scalar2=None, op0=Alu.logical_shift_right)
        nc.vector.tensor_scalar(out=lo_i[:], in0=tok[:], scalar1=255,
                                scalar2=None, op0=Alu.bitwise_and)
        hi_bf = const_tp.tile([P, ROWS, KT], bf16)
        lo_f = const_tp.tile([P, ROWS, KT], f32)
        nlo_f = const_tp.tile([P, ROWS, KT], f32)
        nc.vector.tensor_scalar(out=hi_bf[:], in0=hi_i[:], scalar1=1.0,
                                scalar2=None, op0=Alu.mult)
        nc.vector.tensor_scalar(out=lo_f[:], in0=lo_i[:], scalar1=1.0,
                                scalar2=None, op0=Alu.mult)
        nc.vector.tensor_scalar(out=nlo_f[:], in0=lo_i[:], scalar1=-1.0,
                                scalar2=None, op0=Alu.mult)

        def build_act(r):
            # (iota - lo)^2 then 2^-11 * relu(1 - d2): exact scaled one-hot
            # on the scalar engine
            tiles = {}
            for k in ACT_K:
                sq = sq_tp.tile([P, NLO], bf16, tag="sq")
                ohlo = ohlo_tp.tile([P, NLO], bf16, tag="ohlo_a")
                nc.scalar.activation(
                    out=sq[:], in_=iota_lo[:], func=Act.Square,
                    bias=nlo_f[:, r, k:k + 1], scale=1.0)
                nc.scalar.activation(
                    out=ohlo[:], in_=sq[:], func=Act.Relu,
                    bias=scale_c[:, 0:1], scale=-SCALE)
                tiles[k] = ohlo
            return tiles

        def build_gps(r):
            tiles = {}
            for k in GPS_K:
                ohlo = ohlo_tp.tile([P, NLO], bf16, tag="ohlo_g")
                nc.gpsimd.tensor_scalar(
                    out=ohlo[:], in0=iota_lo[:],
                    scalar1=lo_f[:, r, k:k + 1], scalar2=scale_c[:, 0:1],
                    op0=Alu.is_equal, op1=Alu.mult)
                tiles[k] = ohlo
            return tiles

        def build_hi(r):
            # all 16 k-tiles' hi one-hots in one DVE op
            ohhi16 = ohhi_tp.tile([P, NHI, KT], bf16)
            nc.vector.tensor_tensor(
                out=ohhi16[:], in0=iota_hi16[:],
                in1=hi_bf[:, r, :].unsqueeze(1).broadcast_to([P, NHI, KT]),
                op=Alu.is_equal)
            return ohhi16

        for r in range(ROWS):
            hi_cur = build_hi(r)
            ps = psum_tp.tile([P, NLO], f32)
            for i, k in enumerate(range(KT)):
                if k in ACT_K:
                    sq = sq_tp.tile([P, NLO], bf16, tag="sq")
                    ohlo = ohlo_tp.tile([P, NLO], bf16, tag="ohlo")
                    nc.scalar.activation(
                        out=sq[:], in_=iota_lo[:], func=Act.Square,
                        bias=nlo_f[:, r, k:k + 1], scale=1.0)
                    nc.scalar.activation(
                        out=ohlo[:], in_=sq[:], func=Act.Relu,
                        bias=scale_c[:, 0:1], scale=-SCALE)
                else:
                    ohlo = ohlo_tp.tile([P, NLO], bf16, tag="ohlo")
                    nc.vector.tensor_scalar(
                        out=ohlo[:], in0=iota_lo[:],
                        scalar1=lo_f[:, r, k:k + 1], scalar2=scale_c[:, 0:1],
                        op0=Alu.is_equal, op1=Alu.mult)
                nc.tensor.matmul(out=ps[:], lhsT=hi_cur[:, :, k], rhs=ohlo[:],
                                 start=(i == 0), stop=(i == KT - 1))

            # PSUM holds hist/2048 (gpsimd cannot read PSUM)
            res = res_tp.tile([P, NLO], f32)
            nc.scalar.mul(out=res[:], in_=ps[:], mul=1.0)
            nc.sync.dma_start(
                out=hist[r].rearrange("(h l) -> h l", l=NLO),
                in_=res[:VOCAB // NLO, :])
    nc.compile()
    return nc


_NC_CACHE = None


def _get_nc():
    global _NC_CACHE
    if _NC_CACHE is None:
        _NC_CACHE = _build_nc()
    return _NC_CACHE


def run_sharded(docs: np.ndarray, trace: bool = False):
    """Run the 8-core SPMD kernel. Returns (full_output, BassKernelResults)."""
    from concourse.bass_utils import run_bass_kernel_spmd

    docs = np.ascontiguousarray(np.asarray(docs, dtype=np.int32))
    assert docs.shape == (BATCH, SEQ), docs.shape
    shards = docs.reshape(N_CORES, ROWS, SEQ)
    in_maps = [{"docs": shards[i]} for i in range(N_CORES)]
    res = run_bass_kernel_spmd(_get_nc(), in_maps, core_ids=list(range(N_CORES)),
                               trace=trace)
    out = np.concatenate([res.results[i]["hist"] for i in range(N_CORES)], axis=0)
    return out, res


def kernel(docs: np.ndarray) -> np.ndarray:
    out, _ = run_sharded(docs, trace=False)
    return out
